# revision 8
# baseline (speedup 1.0000x reference)
"""ATSS detection head kernel for 8 Trainium2 NeuronCores.

Sharding: 8 cores = 4 images x {cls tower, bbox tower}. One uniform SPMD
program: each core runs the 4-conv tower + an 85-channel 1x1 head on its
image (cores 0-3 use cls tower weights + wcls head; cores 4-7 use bbox
tower weights + wbox/wctr head rows). Centerness logits are exchanged
between paired cores (i, i+4) with an AllGather so the cls core can score
sigmoid(cls)*sigmoid(ctr) and run a bucketed Max8 top-k pre-selection.
The host gathers head outputs, rescores the pre-selected candidates
bit-exactly with jax-CPU (same backend as the reference), decodes boxes,
and runs the greedy NMS on the top candidates.
"""

import os
import sys
import types

import numpy as np

# ---------------- problem constants (hardcoded) -------------------------
LEVELS = [(100, 100, 8), (50, 50, 16), (25, 25, 32), (13, 13, 64), (7, 7, 128)]
NLVL = len(LEVELS)
NPOS = sum(h * w for h, w, _ in LEVELS)          # 13343
LVL_OFF = np.cumsum([0] + [h * w for h, w, _ in LEVELS]).tolist()
CIN = 256
NCLS = 80
NHEAD = 85                                        # 80 cls | 4 bbox | 1 ctr
NLAYER = 4
NCORES = 8
NIMG = 4
IMG_SIZE = 800.0
PRE_NMS_THRESH = 0.05
NMS_THRESH = 0.6
MAX_DET = 100
SCALE_CLAMP = float(np.log(1000.0 / 16.0))
BBOX_W = (10.0, 10.0, 5.0, 5.0)
TOP_T = 512                                       # NMS candidate window

ROWS_PER_GROUP = [5, 9, 20, 13, 7]                # F = R*W <= 500
BUCKETS = []
for _li, (_h, _w, _s) in enumerate(LEVELS):
    _n = _h * _w
    _off = LVL_OFF[_li]
    _chunk = 1250 if _n >= 1250 else _n
    _p = 0
    while _p < _n:
        _ln = min(_chunk, _n - _p)
        BUCKETS.append((_off + _p, _ln))
        _p += _ln
NBUCK = len(BUCKETS)                              # 13

_BUILT = None


def _install_profile_hook():
    try:
        if 'antenv.axon_hooks' not in sys.modules:
            m = types.ModuleType('antenv.axon_hooks')
            m._hook = None

            def _set(h):
                m._hook = h

            def _get():
                return m._hook

            m.set_axon_ntff_profile_hook = _set
            m.get_axon_ntff_profile_hook = _get
            sys.modules['antenv.axon_hooks'] = m
        import antenv.axon_hooks as ah
        if ah.get_axon_ntff_profile_hook() is None:
            from trn_agent_boot.trn_boot import _ntff_profile_via_ctypes
            ah.set_axon_ntff_profile_hook(
                _ntff_profile_via_ctypes('/opt/axon/libaxon_pjrt.so'))
    except Exception:
        pass


def _build():
    import concourse.bacc as bacc
    import concourse.mybir as mybir
    from concourse.tile import TileContext

    F32 = mybir.dt.float32
    U32 = mybir.dt.uint32
    AF = mybir.ActivationFunctionType

    nc = bacc.Bacc("TRN2", target_bir_lowering=False, debug=False,
                   num_devices=NCORES)

    xin_d = [nc.dram_tensor(f"x{li}", [CIN, h, w], F32, kind="ExternalInput")
             for li, (h, w, _) in enumerate(LEVELS)]
    tw_d = nc.dram_tensor("tw", [NLAYER, 128, 2, 9, 2, 128], F32,
                          kind="ExternalInput")
    tb_d = nc.dram_tensor("tb", [128, NLAYER, 2], F32, kind="ExternalInput")
    wh_d = nc.dram_tensor("wh", [128, 2, NHEAD], F32, kind="ExternalInput")
    hb_d = nc.dram_tensor("hb", [NHEAD, 1], F32, kind="ExternalInput")

    head_d = nc.dram_tensor("head_out", [NHEAD, NPOS], F32,
                            kind="ExternalOutput")
    s1v_d = nc.dram_tensor("s1v", [NCLS, NBUCK, 8], F32, kind="ExternalOutput")
    s1i_d = nc.dram_tensor("s1i", [NCLS, NBUCK, 8], U32, kind="ExternalOutput")

    with TileContext(nc) as tc:
        with tc.tile_pool(name="cons", bufs=1) as cons:
            tb_s = cons.tile([128, NLAYER, 2], F32)
            hb_s = cons.tile([NHEAD, 1], F32)
            nc.sync.dma_start(out=tb_s, in_=tb_d.ap())
            nc.sync.dma_start(out=hb_s, in_=hb_d.ap())

            with tc.tile_pool(name="lvl", bufs=1) as lvlp:
                bufs = []
                for li, (h, w, _) in enumerate(LEVELS):
                    t = lvlp.tile([128, 2, h + 10, w + 2], F32,
                                  name=f"lvl{li}")
                    bufs.append(t)
                    nc.vector.memset(t, 0.0)
                    for c in range(2):
                        nc.sync.dma_start(
                            out=t[:, c, 8:8 + h, 1:1 + w],
                            in_=xin_d[li].ap()[c * 128:(c + 1) * 128, :, :])

                # ---- tower: 4 conv layers, in-place streaming (shift -2) --
                with tc.tile_pool(name="wts", bufs=2) as wtp, \
                     tc.tile_pool(name="pst", bufs=8, space="PSUM") as pst:
                    for layer in range(NLAYER):
                        wt = wtp.tile([128, 2, 9, 2, 128], F32, name="wt",
                                      tag="wt")
                        nc.sync.dma_start(out=wt, in_=tw_d.ap()[layer])
                        f0 = 8 - 2 * layer          # X grid row0 slot
                        f1 = f0 - 2                 # Y grid row0 slot
                        for li, (h, w, _) in enumerate(LEVELS):
                            t = bufs[li]
                            R = ROWS_PER_GROUP[li]
                            y0 = 0
                            while y0 < h:
                                nr = min(R, h - y0)
                                F = nr * w
                                # BOTH co-chunk matmul groups must read the old
                                # X rows before either in-place eviction
                                # overwrites them, so: all matmuls first, then
                                # both evictions (Tile orders via WAR deps).
                                pts = []
                                for coc in range(2):
                                    pt = pst.tile([128, 500], F32,
                                                  name=f"pt{coc}", tag="pt")
                                    pts.append(pt)
                                    k = 0
                                    for tap in range(9):
                                        dy, dx = tap // 3 - 1, tap % 3 - 1
                                        for cic in range(2):
                                            nc.tensor.matmul(
                                                pt[:, :F],
                                                wt[:, cic, tap, coc, :],
                                                t[:, cic,
                                                  f0 + y0 + dy:f0 + y0 + dy + nr,
                                                  1 + dx:1 + dx + w],
                                                start=(k == 0), stop=(k == 17))
                                            k += 1
                                for coc in range(2):
                                    nc.scalar.activation(
                                        t[:, coc, f1 + y0:f1 + y0 + nr, 1:1 + w],
                                        pts[coc][:, :F], AF.Relu,
                                        bias=tb_s[:, layer, coc:coc + 1])
                                y0 += nr
                            if layer < NLAYER - 1:
                                nc.vector.memset(t[:, :, f1 + h, :], 0.0)

                # ---- head: 85-ch 1x1 conv, evicted straight to DRAM ----
                with tc.tile_pool(name="hw", bufs=1) as hwp, \
                     tc.tile_pool(name="bnc", bufs=4) as bnc, \
                     tc.tile_pool(name="psh", bufs=8, space="PSUM") as psh:
                    wh_s = hwp.tile([128, 2, NHEAD], F32)
                    nc.sync.dma_start(out=wh_s, in_=wh_d.ap())
                    for li, (h, w, _) in enumerate(LEVELS):
                        t = bufs[li]
                        R = ROWS_PER_GROUP[li]
                        off = LVL_OFF[li]
                        y0 = 0
                        while y0 < h:
                            nr = min(R, h - y0)
                            F = nr * w
                            ph = psh.tile([NHEAD, 500], F32, name="ph",
                                          tag="ph")
                            for cic in range(2):
                                nc.tensor.matmul(
                                    ph[:, :F], wh_s[:, cic, :],
                                    t[:, cic, y0:y0 + nr, 1:1 + w],
                                    start=(cic == 0), stop=(cic == 1))
                            ho = bnc.tile([NHEAD, 500], F32, name="ho",
                                          tag="ho")
                            nc.scalar.activation(ho[:, :F], ph[:, :F],
                                                 AF.Identity, bias=hb_s)
                            nc.gpsimd.dma_start(
                                out=head_d.ap()[:, off + y0 * w:off + y0 * w + F],
                                in_=ho[:, :F])
                            y0 += nr
            # level buffers released

            # ---- ctr exchange + scoring + stage-1 selection ----
            with tc.tile_pool(name="dramp", bufs=1, space="DRAM") as dpool, \
                 tc.tile_pool(name="sc", bufs=1) as scp, \
                 tc.tile_pool(name="psb", bufs=8, space="PSUM") as psb:
                cc_in = dpool.tile([1, NPOS], F32)
                cc_out = dpool.tile([2, NPOS], F32)
                nc.sync.dma_start(out=cc_in, in_=head_d.ap()[84:85, :])
                nc.gpsimd.collective_compute(
                    "AllGather", mybir.AluOpType.bypass,
                    replica_groups=[[0, 4], [1, 5], [2, 6], [3, 7]],
                    ins=[cc_in.opt()], outs=[cc_out.opt()])
                sctr = scp.tile([1, NPOS], F32)
                p = 0
                while p < NPOS:
                    F = min(4000, NPOS - p)
                    cb = scp.tile([1, 4000], F32, name="cb", tag="cb", bufs=2)
                    nc.sync.dma_start(out=cb[:, :F], in_=cc_out[1:2, p:p + F])
                    nc.scalar.activation(sctr[:, p:p + F], cb[:, :F],
                                         AF.Sigmoid)
                    p += F
                ones80 = scp.tile([1, NCLS], F32)
                nc.vector.memset(ones80, 1.0)
                sctrb = scp.tile([NCLS, NPOS], F32)
                p = 0
                while p < NPOS:
                    F = min(500, NPOS - p)
                    pb = psb.tile([NCLS, 500], F32, name="pb", tag="pb")
                    nc.tensor.matmul(pb[:, :F], ones80, sctr[:, p:p + F],
                                     start=True, stop=True)
                    nc.vector.tensor_copy(sctrb[:, p:p + F], pb[:, :F])
                    p += F
                clst = scp.tile([NCLS, NPOS], F32)
                nc.sync.dma_start(out=clst, in_=head_d.ap()[0:NCLS, :])
                p = 0
                while p < NPOS:
                    F = min(2000, NPOS - p)
                    sb = scp.tile([NCLS, 2000], F32, name="sb", tag="sb",
                                  bufs=2)
                    nc.scalar.activation(sb[:, :F], clst[:, p:p + F],
                                         AF.Sigmoid)
                    nc.vector.tensor_mul(sctrb[:, p:p + F], sctrb[:, p:p + F],
                                         sb[:, :F])
                    p += F
                s1v = scp.tile([NCLS, NBUCK, 8], F32)
                s1i = scp.tile([NCLS, NBUCK, 8], U32)
                for bi, (boff, blen) in enumerate(BUCKETS):
                    nc.vector.max(out=s1v[:, bi, :],
                                  in_=sctrb[:, boff:boff + blen])
                    nc.vector.max_index(out=s1i[:, bi, :],
                                        in_max=s1v[:, bi, :],
                                        in_values=sctrb[:, boff:boff + blen])
                nc.gpsimd.dma_start(out=s1v_d.ap(), in_=s1v)
                nc.gpsimd.dma_start(out=s1i_d.ap(), in_=s1i)

    nc.compile()
    return nc


def _pack_tower(tw):
    # [4, 256co, 256ci, 3, 3] -> [4, 128ci_in, 2cic, 9tap, 2coc, 128co_in]
    t = tw.reshape(NLAYER, 2, 128, 2, 128, 3, 3)
    t = t.transpose(0, 4, 3, 5, 6, 1, 2)
    return np.ascontiguousarray(
        t.reshape(NLAYER, 128, 2, 9, 2, 128), dtype=np.float32)


def _pack_bias(tb):
    t = tb.reshape(NLAYER, 2, 128).transpose(2, 0, 1)
    return np.ascontiguousarray(t, dtype=np.float32)


def _pack_head(wcls, bcls, wbox, bbox_b, wctr, bctr, is_cls):
    wh = np.zeros((NHEAD, CIN), np.float32)
    hb = np.zeros((NHEAD, 1), np.float32)
    if is_cls:
        wh[0:NCLS] = wcls
        hb[0:NCLS, 0] = bcls
    else:
        wh[NCLS:NCLS + 4] = wbox
        hb[NCLS:NCLS + 4, 0] = bbox_b
        wh[NCLS + 4] = wctr[0]
        hb[NCLS + 4, 0] = bctr[0]
    t = wh.reshape(NHEAD, 2, 128).transpose(2, 1, 0)
    return np.ascontiguousarray(t, dtype=np.float32), hb


def _postprocess(cls_heads, bbox_heads, s1i_list, scales):
    import jax
    import jax.numpy as jnp
    cpu = jax.devices("cpu")[0]

    chunk_off = np.array([b[0] for b in BUCKETS], np.int64)
    out_boxes = np.zeros((NIMG, MAX_DET, 4), np.float32)
    out_scores = np.zeros((NIMG, MAX_DET), np.float32)
    out_classes = np.full((NIMG, MAX_DET), -1, np.int32)

    lvl_of = np.zeros(NPOS, np.int32)
    for li in range(NLVL):
        lvl_of[LVL_OFF[li]:LVL_OFF[li + 1]] = li
    stride_of = np.array([s for _, _, s in LEVELS], np.float32)
    w_of = np.array([w for _, w, _ in LEVELS], np.int64)
    off_of = np.array(LVL_OFF[:NLVL], np.int64)

    with jax.default_device(cpu):
        for img in range(NIMG):
            s1i = s1i_list[img].astype(np.int64)          # [80, NBUCK, 8]
            pos = (s1i + chunk_off[None, :, None]).reshape(-1)
            cls = np.repeat(np.arange(NCLS, dtype=np.int64), NBUCK * 8)
            key = cls * NPOS + pos
            _, uniq = np.unique(key, return_index=True)
            pos, cls = pos[uniq], cls[uniq]

            cl = cls_heads[img][cls, pos]
            ct = bbox_heads[img][84, pos]
            lv = lvl_of[pos]

            s = np.asarray(jax.nn.sigmoid(jnp.asarray(cl))
                           * jax.nn.sigmoid(jnp.asarray(ct)))
            s = np.where(s > np.float32(PRE_NMS_THRESH), s, np.float32(0.0))
            sc = np.asarray(
                jnp.sqrt(jnp.where(jnp.asarray(s) > 0, jnp.asarray(s), 1.0))
                * (jnp.asarray(s) > 0))

            flat = (pos - off_of[lv]) * NCLS + cls
            order = np.lexsort((flat, -s.astype(np.float64), lv,
                                -sc.astype(np.float64)))
            order = order[:TOP_T]
            pos_t, cls_t, sc_t, lv_t = (pos[order], cls[order], sc[order],
                                        lv[order])

            stride = stride_of[lv_t]
            wl = w_of[lv_t]
            rel = pos_t - off_of[lv_t]
            xg = rel % wl
            yg = rel // wl
            xs = (jnp.asarray(xg, jnp.float32) + 0.5) * jnp.asarray(stride)
            ys = (jnp.asarray(yg, jnp.float32) + 0.5) * jnp.asarray(stride)
            half = jnp.asarray(stride) * 8.0 / 2.0
            a0, a1, a2, a3 = xs - half, ys - half, xs + half, ys + half
            d_raw = bbox_heads[img][NCLS:NCLS + 4][:, pos_t]
            d = jnp.asarray(d_raw) * jnp.asarray(scales)[lv_t][None, :]
            wx, wy, ww, wh_ = BBOX_W
            aw = a2 - a0
            ah = a3 - a1
            acx = a0 + 0.5 * aw
            acy = a1 + 0.5 * ah
            dx = d[0] / wx
            dy = d[1] / wy
            dw = jnp.minimum(d[2] / ww, SCALE_CLAMP)
            dh = jnp.minimum(d[3] / wh_, SCALE_CLAMP)
            pcx = dx * aw + acx
            pcy = dy * ah + acy
            pw = jnp.exp(dw) * aw
            ph = jnp.exp(dh) * ah
            boxes = jnp.stack([pcx - 0.5 * pw, pcy - 0.5 * ph,
                               pcx + 0.5 * pw, pcy + 0.5 * ph], -1)
            boxes = np.asarray(jnp.clip(boxes, 0.0, IMG_SIZE), np.float32)

            offv = cls_t.astype(np.float32) * np.float32(2.0 * IMG_SIZE)
            ob = boxes + offv[:, None]
            s_run = sc_t.astype(np.float32).copy()
            a_all = (ob[:, 2] - ob[:, 0]) * (ob[:, 3] - ob[:, 1])
            for it in range(MAX_DET):
                i = int(np.argmax(s_run))
                best = s_run[i]
                x1 = np.maximum(ob[i, 0], ob[:, 0])
                y1 = np.maximum(ob[i, 1], ob[:, 1])
                x2 = np.minimum(ob[i, 2], ob[:, 2])
                y2 = np.minimum(ob[i, 3], ob[:, 3])
                inter = (np.maximum(x2 - x1, np.float32(0.0))
                         * np.maximum(y2 - y1, np.float32(0.0)))
                a1_ = (ob[i, 2] - ob[i, 0]) * (ob[i, 3] - ob[i, 1])
                iou = inter / (a1_ + a_all - inter + np.float32(1e-9))
                s_run = np.where(iou > np.float32(NMS_THRESH),
                                 np.float32(-1.0), s_run)
                s_run[i] = np.float32(-1.0)
                if best > 0:
                    out_boxes[img, it] = boxes[i]
                    out_scores[img, it] = best
                    out_classes[img, it] = cls_t[i]
    return out_boxes, out_scores, out_classes


def kernel(p3, p4, p5, p6, p7, cls_tw, cls_tb, bbox_tw, bbox_tb,
           wcls, bcls, wbox, bbox_b, wctr, bctr, scales):
    global _BUILT
    _install_profile_hook()
    from concourse import bass_utils
    bass_utils.upload_artifacts = lambda tmpdir: tmpdir

    if _BUILT is None:
        _BUILT = _build()
    nc = _BUILT

    feats = [np.asarray(p3), np.asarray(p4), np.asarray(p5),
             np.asarray(p6), np.asarray(p7)]
    cls_pack = _pack_tower(np.asarray(cls_tw))
    bbox_pack = _pack_tower(np.asarray(bbox_tw))
    cls_bias = _pack_bias(np.asarray(cls_tb))
    bbox_bias = _pack_bias(np.asarray(bbox_tb))
    whc, hbc = _pack_head(np.asarray(wcls), np.asarray(bcls), np.asarray(wbox),
                          np.asarray(bbox_b), np.asarray(wctr),
                          np.asarray(bctr), True)
    whb, hbb = _pack_head(np.asarray(wcls), np.asarray(bcls), np.asarray(wbox),
                          np.asarray(bbox_b), np.asarray(wctr),
                          np.asarray(bctr), False)

    in_maps = []
    for core in range(NCORES):
        img = core % NIMG
        is_cls = core < NIMG
        m = {}
        for li in range(NLVL):
            m[f"x{li}"] = np.ascontiguousarray(feats[li][img])
        m["tw"] = cls_pack if is_cls else bbox_pack
        m["tb"] = cls_bias if is_cls else bbox_bias
        m["wh"] = whc if is_cls else whb
        m["hb"] = hbc if is_cls else hbb
        in_maps.append(m)

    trace = bool(os.environ.get("KERNEL_PROFILE"))
    try:
        res = bass_utils.run_bass_kernel_spmd(
            nc, in_maps, core_ids=list(range(NCORES)), trace=trace)
    except Exception:
        # device may be wedged from a prior aborted run; reset and retry once
        try:
            import ctypes
            import jax as _jax
            _jax.devices()
            ctypes.CDLL('/opt/axon/libaxon_pjrt.so').axon_reset()
        except Exception:
            pass
        res = bass_utils.run_bass_kernel_spmd(
            nc, in_maps, core_ids=list(range(NCORES)), trace=trace)
    if trace:
        print(f"HW exec time: {res.exec_time_ns} ns")
        kernel.last_exec_time_ns = res.exec_time_ns

    cls_heads = [res.results[i]["head_out"] for i in range(NIMG)]
    bbox_heads = [res.results[i + NIMG]["head_out"] for i in range(NIMG)]
    s1i_list = [res.results[i]["s1i"] for i in range(NIMG)]
    kernel.last_raw = (cls_heads, bbox_heads, s1i_list,
                       [res.results[i]["s1v"] for i in range(NIMG)])
    return _postprocess(cls_heads, bbox_heads, s1i_list,
                        np.asarray(scales, np.float32))


# revision 9
# speedup vs baseline: 1.1953x; 1.1953x over previous
"""ATSS detection head kernel for 8 Trainium2 NeuronCores.

Sharding: 8 cores = 4 images x {cls tower, bbox tower}. One uniform SPMD
program: each core runs the 4-conv tower + an 85-channel 1x1 head on its
image (cores 0-3 use cls tower weights + wcls head; cores 4-7 use bbox
tower weights + wbox/wctr head rows). Centerness logits are exchanged
between paired cores (i, i+4) with an AllGather so the cls core can score
sigmoid(cls)*sigmoid(ctr) and run a bucketed Max8 top-k pre-selection.
The host gathers head outputs, rescores the pre-selected candidates
bit-exactly with jax-CPU (same backend as the reference), decodes boxes,
and runs the greedy NMS on the top candidates.
"""

import os
import sys
import types

import numpy as np

# ---------------- problem constants (hardcoded) -------------------------
LEVELS = [(100, 100, 8), (50, 50, 16), (25, 25, 32), (13, 13, 64), (7, 7, 128)]
NLVL = len(LEVELS)
NPOS = sum(h * w for h, w, _ in LEVELS)          # 13343
LVL_OFF = np.cumsum([0] + [h * w for h, w, _ in LEVELS]).tolist()
CIN = 256
NCLS = 80
NHEAD = 85                                        # 80 cls | 4 bbox | 1 ctr
NLAYER = 4
NCORES = 8
NIMG = 4
IMG_SIZE = 800.0
PRE_NMS_THRESH = 0.05
NMS_THRESH = 0.6
MAX_DET = 100
SCALE_CLAMP = float(np.log(1000.0 / 16.0))
BBOX_W = (10.0, 10.0, 5.0, 5.0)
TOP_T = 512                                       # NMS candidate window

ROWS_PER_GROUP = [5, 10, 20, 13, 7]                # F = R*W <= 500
BUCKETS = []
for _li, (_h, _w, _s) in enumerate(LEVELS):
    _n = _h * _w
    _off = LVL_OFF[_li]
    _chunk = 1250 if _n >= 1250 else _n
    _p = 0
    while _p < _n:
        _ln = min(_chunk, _n - _p)
        BUCKETS.append((_off + _p, _ln))
        _p += _ln
NBUCK = len(BUCKETS)                              # 13

_BUILT = None


def _install_profile_hook():
    try:
        if 'antenv.axon_hooks' not in sys.modules:
            m = types.ModuleType('antenv.axon_hooks')
            m._hook = None

            def _set(h):
                m._hook = h

            def _get():
                return m._hook

            m.set_axon_ntff_profile_hook = _set
            m.get_axon_ntff_profile_hook = _get
            sys.modules['antenv.axon_hooks'] = m
        import antenv.axon_hooks as ah
        if ah.get_axon_ntff_profile_hook() is None:
            from trn_agent_boot.trn_boot import _ntff_profile_via_ctypes
            ah.set_axon_ntff_profile_hook(
                _ntff_profile_via_ctypes('/opt/axon/libaxon_pjrt.so'))
    except Exception:
        pass


def _build():
    import concourse.bacc as bacc
    import concourse.mybir as mybir
    from concourse.tile import TileContext

    F32 = mybir.dt.float32
    U32 = mybir.dt.uint32
    AF = mybir.ActivationFunctionType

    nc = bacc.Bacc("TRN2", target_bir_lowering=False, debug=False,
                   num_devices=NCORES)

    xin_d = [nc.dram_tensor(f"x{li}", [CIN, h, w], F32, kind="ExternalInput")
             for li, (h, w, _) in enumerate(LEVELS)]
    tw_d = nc.dram_tensor("tw", [NLAYER, 128, 2, 9, 2, 128], F32,
                          kind="ExternalInput")
    tb_d = nc.dram_tensor("tb", [128, NLAYER, 2], F32, kind="ExternalInput")
    wh_d = nc.dram_tensor("wh", [128, 2, NHEAD], F32, kind="ExternalInput")
    hb_d = nc.dram_tensor("hb", [NHEAD, 1], F32, kind="ExternalInput")

    head_d = nc.dram_tensor("head_out", [NHEAD, NPOS], F32,
                            kind="ExternalOutput")
    s1v_d = nc.dram_tensor("s1v", [NCLS, NBUCK, 8], F32, kind="ExternalOutput")
    s1i_d = nc.dram_tensor("s1i", [NCLS, NBUCK, 8], U32, kind="ExternalOutput")

    with TileContext(nc) as tc:
        with tc.tile_pool(name="cons", bufs=1) as cons:
            tb_s = cons.tile([128, NLAYER, 2], F32)
            hb_s = cons.tile([NHEAD, 1], F32)
            nc.sync.dma_start(out=tb_s, in_=tb_d.ap())
            nc.sync.dma_start(out=hb_s, in_=hb_d.ap())

            with tc.tile_pool(name="lvl", bufs=1) as lvlp:
                bufs = []
                for li, (h, w, _) in enumerate(LEVELS):
                    t = lvlp.tile([128, 2, h + 10, w + 2], F32,
                                  name=f"lvl{li}")
                    bufs.append(t)
                    nc.vector.memset(t, 0.0)
                    for c in range(2):
                        nc.sync.dma_start(
                            out=t[:, c, 8:8 + h, 1:1 + w],
                            in_=xin_d[li].ap()[c * 128:(c + 1) * 128, :, :])

                # ---- tower: 4 conv layers, in-place streaming (shift -2) --
                with tc.tile_pool(name="wts", bufs=2) as wtp, \
                     tc.tile_pool(name="pst", bufs=8, space="PSUM") as pst:
                    for layer in range(NLAYER):
                        wt = wtp.tile([128, 2, 9, 2, 128], F32, name="wt",
                                      tag="wt")
                        nc.sync.dma_start(out=wt, in_=tw_d.ap()[layer])
                        f0 = 8 - 2 * layer          # X grid row0 slot
                        f1 = f0 - 2                 # Y grid row0 slot
                        for li, (h, w, _) in enumerate(LEVELS):
                            t = bufs[li]
                            R = ROWS_PER_GROUP[li]
                            y0 = 0
                            while y0 < h:
                                nr = min(R, h - y0)
                                F = nr * w
                                # BOTH co-chunk matmul groups must read the old
                                # X rows before either in-place eviction
                                # overwrites them, so: all matmuls first, then
                                # both evictions (Tile orders via WAR deps).
                                pts = []
                                for coc in range(2):
                                    pt = pst.tile([128, 500], F32,
                                                  name=f"pt{coc}", tag="pt")
                                    pts.append(pt)
                                    k = 0
                                    for tap in range(9):
                                        dy, dx = tap // 3 - 1, tap % 3 - 1
                                        for cic in range(2):
                                            nc.tensor.matmul(
                                                pt[:, :F],
                                                wt[:, cic, tap, coc, :],
                                                t[:, cic,
                                                  f0 + y0 + dy:f0 + y0 + dy + nr,
                                                  1 + dx:1 + dx + w],
                                                start=(k == 0), stop=(k == 17))
                                            k += 1
                                for coc in range(2):
                                    nc.scalar.activation(
                                        t[:, coc, f1 + y0:f1 + y0 + nr, 1:1 + w],
                                        pts[coc][:, :F], AF.Relu,
                                        bias=tb_s[:, layer, coc:coc + 1])
                                y0 += nr
                            if layer < NLAYER - 1:
                                nc.vector.memset(t[:, :, f1 + h, :], 0.0)

                # ---- head: 85-ch 1x1 conv, evicted straight to DRAM ----
                with tc.tile_pool(name="hw", bufs=1) as hwp, \
                     tc.tile_pool(name="bnc", bufs=4) as bnc, \
                     tc.tile_pool(name="psh", bufs=8, space="PSUM") as psh:
                    wh_s = hwp.tile([128, 2, NHEAD], F32)
                    nc.sync.dma_start(out=wh_s, in_=wh_d.ap())
                    for li, (h, w, _) in enumerate(LEVELS):
                        t = bufs[li]
                        R = ROWS_PER_GROUP[li]
                        off = LVL_OFF[li]
                        y0 = 0
                        while y0 < h:
                            nr = min(R, h - y0)
                            F = nr * w
                            ph = psh.tile([NHEAD, 500], F32, name="ph",
                                          tag="ph")
                            for cic in range(2):
                                nc.tensor.matmul(
                                    ph[:, :F], wh_s[:, cic, :],
                                    t[:, cic, y0:y0 + nr, 1:1 + w],
                                    start=(cic == 0), stop=(cic == 1))
                            ho = bnc.tile([NHEAD, 500], F32, name="ho",
                                          tag="ho")
                            nc.scalar.activation(ho[:, :F], ph[:, :F],
                                                 AF.Identity, bias=hb_s)
                            nc.sync.dma_start(
                                out=head_d.ap()[:, off + y0 * w:off + y0 * w + F],
                                in_=ho[:, :F])
                            y0 += nr
            # level buffers released

            # ---- ctr exchange + scoring + stage-1 selection ----
            with tc.tile_pool(name="dramp", bufs=1, space="DRAM") as dpool, \
                 tc.tile_pool(name="sc", bufs=1) as scp, \
                 tc.tile_pool(name="psb", bufs=8, space="PSUM") as psb:
                cc_in = dpool.tile([1, NPOS], F32)
                cc_out = dpool.tile([2, NPOS], F32)
                nc.sync.dma_start(out=cc_in, in_=head_d.ap()[84:85, :])
                nc.gpsimd.collective_compute(
                    "AllGather", mybir.AluOpType.bypass,
                    replica_groups=[[0, 4], [1, 5], [2, 6], [3, 7]],
                    ins=[cc_in.opt()], outs=[cc_out.opt()])
                sctr = scp.tile([1, NPOS], F32)
                p = 0
                while p < NPOS:
                    F = min(4000, NPOS - p)
                    cb = scp.tile([1, 4000], F32, name="cb", tag="cb", bufs=2)
                    nc.sync.dma_start(out=cb[:, :F], in_=cc_out[1:2, p:p + F])
                    nc.scalar.activation(sctr[:, p:p + F], cb[:, :F],
                                         AF.Sigmoid)
                    p += F
                ones80 = scp.tile([1, NCLS], F32)
                nc.vector.memset(ones80, 1.0)
                sctrb = scp.tile([NCLS, NPOS], F32)
                p = 0
                while p < NPOS:
                    F = min(500, NPOS - p)
                    pb = psb.tile([NCLS, 500], F32, name="pb", tag="pb")
                    nc.tensor.matmul(pb[:, :F], ones80, sctr[:, p:p + F],
                                     start=True, stop=True)
                    nc.vector.tensor_copy(sctrb[:, p:p + F], pb[:, :F])
                    p += F
                clst = scp.tile([NCLS, NPOS], F32)
                nc.sync.dma_start(out=clst, in_=head_d.ap()[0:NCLS, :])
                p = 0
                while p < NPOS:
                    F = min(2000, NPOS - p)
                    sb = scp.tile([NCLS, 2000], F32, name="sb", tag="sb",
                                  bufs=2)
                    nc.scalar.activation(sb[:, :F], clst[:, p:p + F],
                                         AF.Sigmoid)
                    nc.vector.tensor_mul(sctrb[:, p:p + F], sctrb[:, p:p + F],
                                         sb[:, :F])
                    p += F
                s1v = scp.tile([NCLS, NBUCK, 8], F32)
                s1i = scp.tile([NCLS, NBUCK, 8], U32)
                for bi, (boff, blen) in enumerate(BUCKETS):
                    nc.vector.max(out=s1v[:, bi, :],
                                  in_=sctrb[:, boff:boff + blen])
                    nc.vector.max_index(out=s1i[:, bi, :],
                                        in_max=s1v[:, bi, :],
                                        in_values=sctrb[:, boff:boff + blen])
                nc.sync.dma_start(out=s1v_d.ap(), in_=s1v)
                nc.sync.dma_start(out=s1i_d.ap(), in_=s1i)

    nc.compile()
    return nc


def _pack_tower(tw):
    # [4, 256co, 256ci, 3, 3] -> [4, 128ci_in, 2cic, 9tap, 2coc, 128co_in]
    t = tw.reshape(NLAYER, 2, 128, 2, 128, 3, 3)
    t = t.transpose(0, 4, 3, 5, 6, 1, 2)
    return np.ascontiguousarray(
        t.reshape(NLAYER, 128, 2, 9, 2, 128), dtype=np.float32)


def _pack_bias(tb):
    t = tb.reshape(NLAYER, 2, 128).transpose(2, 0, 1)
    return np.ascontiguousarray(t, dtype=np.float32)


def _pack_head(wcls, bcls, wbox, bbox_b, wctr, bctr, is_cls):
    wh = np.zeros((NHEAD, CIN), np.float32)
    hb = np.zeros((NHEAD, 1), np.float32)
    if is_cls:
        wh[0:NCLS] = wcls
        hb[0:NCLS, 0] = bcls
    else:
        wh[NCLS:NCLS + 4] = wbox
        hb[NCLS:NCLS + 4, 0] = bbox_b
        wh[NCLS + 4] = wctr[0]
        hb[NCLS + 4, 0] = bctr[0]
    t = wh.reshape(NHEAD, 2, 128).transpose(2, 1, 0)
    return np.ascontiguousarray(t, dtype=np.float32), hb


def _postprocess(cls_heads, bbox_heads, s1i_list, scales):
    import jax
    import jax.numpy as jnp
    cpu = jax.devices("cpu")[0]

    chunk_off = np.array([b[0] for b in BUCKETS], np.int64)
    out_boxes = np.zeros((NIMG, MAX_DET, 4), np.float32)
    out_scores = np.zeros((NIMG, MAX_DET), np.float32)
    out_classes = np.full((NIMG, MAX_DET), -1, np.int32)

    lvl_of = np.zeros(NPOS, np.int32)
    for li in range(NLVL):
        lvl_of[LVL_OFF[li]:LVL_OFF[li + 1]] = li
    stride_of = np.array([s for _, _, s in LEVELS], np.float32)
    w_of = np.array([w for _, w, _ in LEVELS], np.int64)
    off_of = np.array(LVL_OFF[:NLVL], np.int64)

    with jax.default_device(cpu):
        for img in range(NIMG):
            s1i = s1i_list[img].astype(np.int64)          # [80, NBUCK, 8]
            pos = (s1i + chunk_off[None, :, None]).reshape(-1)
            cls = np.repeat(np.arange(NCLS, dtype=np.int64), NBUCK * 8)
            key = cls * NPOS + pos
            _, uniq = np.unique(key, return_index=True)
            pos, cls = pos[uniq], cls[uniq]

            cl = cls_heads[img][cls, pos]
            ct = bbox_heads[img][84, pos]
            lv = lvl_of[pos]

            s = np.asarray(jax.nn.sigmoid(jnp.asarray(cl))
                           * jax.nn.sigmoid(jnp.asarray(ct)))
            s = np.where(s > np.float32(PRE_NMS_THRESH), s, np.float32(0.0))
            sc = np.asarray(
                jnp.sqrt(jnp.where(jnp.asarray(s) > 0, jnp.asarray(s), 1.0))
                * (jnp.asarray(s) > 0))

            flat = (pos - off_of[lv]) * NCLS + cls
            order = np.lexsort((flat, -s.astype(np.float64), lv,
                                -sc.astype(np.float64)))
            order = order[:TOP_T]
            pos_t, cls_t, sc_t, lv_t = (pos[order], cls[order], sc[order],
                                        lv[order])

            stride = stride_of[lv_t]
            wl = w_of[lv_t]
            rel = pos_t - off_of[lv_t]
            xg = rel % wl
            yg = rel // wl
            xs = (jnp.asarray(xg, jnp.float32) + 0.5) * jnp.asarray(stride)
            ys = (jnp.asarray(yg, jnp.float32) + 0.5) * jnp.asarray(stride)
            half = jnp.asarray(stride) * 8.0 / 2.0
            a0, a1, a2, a3 = xs - half, ys - half, xs + half, ys + half
            d_raw = bbox_heads[img][NCLS:NCLS + 4][:, pos_t]
            d = jnp.asarray(d_raw) * jnp.asarray(scales)[lv_t][None, :]
            wx, wy, ww, wh_ = BBOX_W
            aw = a2 - a0
            ah = a3 - a1
            acx = a0 + 0.5 * aw
            acy = a1 + 0.5 * ah
            dx = d[0] / wx
            dy = d[1] / wy
            dw = jnp.minimum(d[2] / ww, SCALE_CLAMP)
            dh = jnp.minimum(d[3] / wh_, SCALE_CLAMP)
            pcx = dx * aw + acx
            pcy = dy * ah + acy
            pw = jnp.exp(dw) * aw
            ph = jnp.exp(dh) * ah
            boxes = jnp.stack([pcx - 0.5 * pw, pcy - 0.5 * ph,
                               pcx + 0.5 * pw, pcy + 0.5 * ph], -1)
            boxes = np.asarray(jnp.clip(boxes, 0.0, IMG_SIZE), np.float32)

            offv = cls_t.astype(np.float32) * np.float32(2.0 * IMG_SIZE)
            ob = boxes + offv[:, None]
            s_run = sc_t.astype(np.float32).copy()
            a_all = (ob[:, 2] - ob[:, 0]) * (ob[:, 3] - ob[:, 1])
            for it in range(MAX_DET):
                i = int(np.argmax(s_run))
                best = s_run[i]
                x1 = np.maximum(ob[i, 0], ob[:, 0])
                y1 = np.maximum(ob[i, 1], ob[:, 1])
                x2 = np.minimum(ob[i, 2], ob[:, 2])
                y2 = np.minimum(ob[i, 3], ob[:, 3])
                inter = (np.maximum(x2 - x1, np.float32(0.0))
                         * np.maximum(y2 - y1, np.float32(0.0)))
                a1_ = (ob[i, 2] - ob[i, 0]) * (ob[i, 3] - ob[i, 1])
                iou = inter / (a1_ + a_all - inter + np.float32(1e-9))
                s_run = np.where(iou > np.float32(NMS_THRESH),
                                 np.float32(-1.0), s_run)
                s_run[i] = np.float32(-1.0)
                if best > 0:
                    out_boxes[img, it] = boxes[i]
                    out_scores[img, it] = best
                    out_classes[img, it] = cls_t[i]
    return out_boxes, out_scores, out_classes


def kernel(p3, p4, p5, p6, p7, cls_tw, cls_tb, bbox_tw, bbox_tb,
           wcls, bcls, wbox, bbox_b, wctr, bctr, scales):
    global _BUILT
    _install_profile_hook()
    from concourse import bass_utils
    bass_utils.upload_artifacts = lambda tmpdir: tmpdir

    if _BUILT is None:
        _BUILT = _build()
    nc = _BUILT

    feats = [np.asarray(p3), np.asarray(p4), np.asarray(p5),
             np.asarray(p6), np.asarray(p7)]
    cls_pack = _pack_tower(np.asarray(cls_tw))
    bbox_pack = _pack_tower(np.asarray(bbox_tw))
    cls_bias = _pack_bias(np.asarray(cls_tb))
    bbox_bias = _pack_bias(np.asarray(bbox_tb))
    whc, hbc = _pack_head(np.asarray(wcls), np.asarray(bcls), np.asarray(wbox),
                          np.asarray(bbox_b), np.asarray(wctr),
                          np.asarray(bctr), True)
    whb, hbb = _pack_head(np.asarray(wcls), np.asarray(bcls), np.asarray(wbox),
                          np.asarray(bbox_b), np.asarray(wctr),
                          np.asarray(bctr), False)

    in_maps = []
    for core in range(NCORES):
        img = core % NIMG
        is_cls = core < NIMG
        m = {}
        for li in range(NLVL):
            m[f"x{li}"] = np.ascontiguousarray(feats[li][img])
        m["tw"] = cls_pack if is_cls else bbox_pack
        m["tb"] = cls_bias if is_cls else bbox_bias
        m["wh"] = whc if is_cls else whb
        m["hb"] = hbc if is_cls else hbb
        in_maps.append(m)

    trace = bool(os.environ.get("KERNEL_PROFILE"))
    try:
        res = bass_utils.run_bass_kernel_spmd(
            nc, in_maps, core_ids=list(range(NCORES)), trace=trace)
    except Exception:
        # device may be wedged from a prior aborted run; reset and retry once
        try:
            import ctypes
            import jax as _jax
            _jax.devices()
            ctypes.CDLL('/opt/axon/libaxon_pjrt.so').axon_reset()
        except Exception:
            pass
        res = bass_utils.run_bass_kernel_spmd(
            nc, in_maps, core_ids=list(range(NCORES)), trace=trace)
    if trace:
        print(f"HW exec time: {res.exec_time_ns} ns")
        kernel.last_exec_time_ns = res.exec_time_ns

    cls_heads = [res.results[i]["head_out"] for i in range(NIMG)]
    bbox_heads = [res.results[i + NIMG]["head_out"] for i in range(NIMG)]
    s1i_list = [res.results[i]["s1i"] for i in range(NIMG)]
    kernel.last_raw = (cls_heads, bbox_heads, s1i_list,
                       [res.results[i]["s1v"] for i in range(NIMG)])
    return _postprocess(cls_heads, bbox_heads, s1i_list,
                        np.asarray(scales, np.float32))
